# revision 1
# baseline (speedup 1.0000x reference)
"""Cross-attention kernel for Trainium2 (8 NeuronCores, SPMD data-parallel).

Problem: O = softmax(Q @ K^T) @ V with B=4, Lq=Lk=4096, D=64, fp32 (no
1/sqrt(d) scaling).

Sharding: 8 cores = 4 batches x 2 Lq-halves. Each core handles a
[2048, 64] Q shard against the full [4096, 64] K/V of its batch.
Independent outputs -> no collectives.

Per-core algorithm (layouts chosen so nothing is transposed on-chip):
  - Host supplies QT [64, 2048] / KT [64, 4096] in fp16 (D on partitions),
    duplicated on-chip across both partition halves so two k-chunks'
    score matmuls run concurrently in the PE array via row tiling
    (contraction is only 64 rows deep).
  - ST[k, q] = matmul(lhsT=KT chunk [64,128], rhs=QT [64,512]) -> PSUM.
  - PT = exp(ST) on the scalar engine, written as bf16 (no max
    subtraction: |scores| < ~50, exp fits fp32/bf16 range; fp16 P would
    underflow).  The scalar engine at 1 elem/cycle/lane is the kernel's
    bottleneck, so exp instructions are kept at 1024 free elements.
  - OT[65, q] += matmul(lhsT=VA chunk [128, 65] bf16, rhs=PT [128, 512]):
    VA = concat([V, ones], 1); rows 0..63 accumulate unnormalized output,
    row 64 the softmax denominator. PT is consumed directly as lhsT-free
    rhs - no transpose anywhere.
  - Normalize: fast-reciprocal of row 64, gpsimd partition-broadcast,
    multiply, DMA out OT [64, 2048]; host transposes back.
"""

import sys

for _p in ("/opt/trn_rl_repo", "/opt/pypackages"):
    if _p not in sys.path:
        sys.path.insert(0, _p)

from contextlib import ExitStack

import ml_dtypes
import numpy as np

import concourse.bacc as bacc
import concourse.mybir as mybir
import concourse.tile as tile
from concourse.bass_utils import run_bass_kernel_spmd

# Problem constants (hardcoded per contract).
B, LQ, LK, D = 4, 4096, 4096, 64
N_CORES = 8
LQ_SHARD = LQ * B // N_CORES  # 2048
QB = 1024  # q-block (exp instruction free-size; 2 PSUM banks)
NQB = LQ_SHARD // QB  # 2
KC = 128  # k-chunk (contraction tile for the PV matmul)
NKC = LK // KC  # 32
SL = 512  # matmul moving-dim slice (one PSUM bank)
NSL = QB // SL  # 2

F32 = mybir.dt.float32
F16 = mybir.dt.float16
BF16 = mybir.dt.bfloat16

BF16NP = ml_dtypes.bfloat16

PACK_S = True  # row-tile two k-chunks' score matmuls concurrently
FAST_RECIP = True  # approx+NR reciprocal (~2 ULP) instead of exact (~6.5us)

KT_PIECE = 512  # kt DMA piece width (cols); 4 k-chunks per piece
VA_PIECE = 8  # va DMA piece size in k-chunks


def _build_program():
    nc = bacc.Bacc(
        "TRN2",
        target_bir_lowering=False,
        debug=False,
        num_devices=N_CORES,
    )
    qt_d = nc.declare_dram_parameter("QT", [D, LQ_SHARD], F16, isOutput=False)
    kt_d = nc.declare_dram_parameter("KT", [D, LK], F16, isOutput=False)
    va_d = nc.declare_dram_parameter("VA", [LK, D + 1], BF16, isOutput=False)
    ot_d = nc.declare_dram_parameter("OT", [D, LQ_SHARD], F32, isOutput=True)

    with tile.TileContext(nc) as tc, ExitStack() as ctx:
        singles = ctx.enter_context(tc.tile_pool(name="singles", bufs=1))
        st_pool = ctx.enter_context(tc.tile_pool(name="st", bufs=2, space="PSUM"))
        ot_pool = ctx.enter_context(tc.tile_pool(name="ot", bufs=2, space="PSUM"))
        pt_pool = ctx.enter_context(tc.tile_pool(name="pt", bufs=3))
        out_pool = ctx.enter_context(tc.tile_pool(name="out", bufs=2))
        norm_pool = ctx.enter_context(tc.tile_pool(name="norm", bufs=4))

        # Preload the exp activation table while input DMAs run.
        warm = singles.tile([1, 2], F32)
        nc.vector.memset(warm[:, :], 0.0)
        nc.scalar.activation(
            out=warm[:, :], in_=warm[:, :],
            func=mybir.ActivationFunctionType.Exp,
        )

        # QT/KT duplicated across both partition halves for PE row tiling.
        # Inputs are split into halves (separate tiles) so the first score
        # matmuls don't wait for the full 2 MB of loads; keeping the piece
        # count low preserves the Tile scheduler's pairing of the row-tiled
        # matmuls (many small tiles reorder the PE stream and let HAM
        # re-throttle the PE clock).
        va_r = va_d[:, :].rearrange("(c p) d -> p c d", p=KC)
        KH = LK // 2  # kt half width
        VH = NKC // 2  # va half size in chunks
        kt_sb = []
        qt_sb = []
        va_sb = []
        for h in range(2):
            tq = singles.tile([2 * D, QB], F16, name=f"qt{h}")
            sq = slice(h * QB, (h + 1) * QB)
            nc.sync.dma_start(out=tq[0:D, :], in_=qt_d[:, sq])
            nc.sync.dma_start(out=tq[D : 2 * D, :], in_=qt_d[:, sq])
            qt_sb.append(tq)
            t = singles.tile([2 * D, KH], F16, name=f"kt{h}")
            sl = slice(h * KH, (h + 1) * KH)
            nc.sync.dma_start(out=t[0:D, :], in_=kt_d[:, sl])
            nc.sync.dma_start(out=t[D : 2 * D, :], in_=kt_d[:, sl])
            kt_sb.append(t)
            tv = singles.tile([KC, VH, D + 1], BF16, name=f"va{h}")
            nc.sync.dma_start(
                out=tv[:, :, :], in_=va_r[:, h * VH : (h + 1) * VH, :]
            )
            va_sb.append(tv)

        def kt_ap(half, c):
            # [64, 128] fp16 weights for chunk c from partition half `half`
            t = kt_sb[c * KC // KH]
            off = (c * KC) % KH
            return t[half * D : (half + 1) * D, off : off + KC]

        def va_ap(c):
            return va_sb[c // VH][:, c % VH, :]

        for qb in range(NQB):
            ot_ps = ot_pool.tile([D + 1, QB], F32)
            for cp in range(NKC // 2):  # chunk pairs, row-tiled in the PE
                c0, c1 = 2 * cp, 2 * cp + 1
                st_a = st_pool.tile([KC, QB], F32, tag="st")
                st_b = st_pool.tile([KC, QB], F32, tag="st")
                for s in range(NSL):
                    q0 = qb * QB + s * SL
                    qt = qt_sb[qb]
                    if PACK_S:
                        nc.tensor.matmul(
                            out=st_a[:, s * SL : (s + 1) * SL],
                            lhsT=kt_ap(0, c0),
                            rhs=qt[0:D, s * SL : (s + 1) * SL],
                            start=True,
                            stop=True,
                            tile_position=(0, 0),
                        )
                        nc.tensor.matmul(
                            out=st_b[:, s * SL : (s + 1) * SL],
                            lhsT=kt_ap(1, c1),
                            rhs=qt[D : 2 * D, s * SL : (s + 1) * SL],
                            start=True,
                            stop=True,
                            tile_position=(D, 0),
                        )
                    else:
                        nc.tensor.matmul(
                            out=st_a[:, s * SL : (s + 1) * SL],
                            lhsT=kt_ap(0, c0),
                            rhs=qt[0:D, s * SL : (s + 1) * SL],
                            start=True,
                            stop=True,
                        )
                        nc.tensor.matmul(
                            out=st_b[:, s * SL : (s + 1) * SL],
                            lhsT=kt_ap(0, c1),
                            rhs=qt[0:D, s * SL : (s + 1) * SL],
                            start=True,
                            stop=True,
                        )
                for c, st_ps in ((c0, st_a), (c1, st_b)):
                    pt = pt_pool.tile([KC, QB], BF16)
                    nc.scalar.activation(
                        out=pt[:, :],
                        in_=st_ps[:, :],
                        func=mybir.ActivationFunctionType.Exp,
                    )
                    for s in range(NSL):
                        nc.tensor.matmul(
                            out=ot_ps[:, s * SL : (s + 1) * SL],
                            lhsT=va_ap(c),
                            rhs=pt[:, s * SL : (s + 1) * SL],
                            start=(c == 0),
                            stop=(c == NKC - 1),
                        )
            # Normalize: O[d, q] = OT[d, q] / OT[64, q]
            recip = norm_pool.tile([1, QB], F32)
            if FAST_RECIP:
                den = norm_pool.tile([1, QB], F32)
                nc.vector.tensor_copy(den[:, :], ot_ps[D : D + 1, :])
                scratch = norm_pool.tile([1, QB], F32)
                nc.vector.reciprocal_approx_accurate(
                    recip[:, :], den[:, :], scratch[:, :]
                )
            else:
                nc.vector.reciprocal(out=recip[:, :], in_=ot_ps[D : D + 1, :])
            bcast = norm_pool.tile([D, QB], F32)
            nc.gpsimd.partition_broadcast(bcast[:, :], recip[:, :])
            o_sb = out_pool.tile([D, QB], F32)
            nc.vector.tensor_mul(o_sb[:, :], ot_ps[0:D, :], bcast[:, :])
            nc.sync.dma_start(
                out=ot_d[:, qb * QB : (qb + 1) * QB], in_=o_sb[:, :]
            )

    nc.finalize()
    return nc


_PROGRAM_CACHE = {}


def _get_program():
    if "nc" not in _PROGRAM_CACHE:
        _PROGRAM_CACHE["nc"] = _build_program()
    return _PROGRAM_CACHE["nc"]


def _make_in_maps(Q, K, V):
    Q = np.asarray(Q, dtype=np.float32)
    K = np.asarray(K, dtype=np.float32)
    V = np.asarray(V, dtype=np.float32)
    in_maps = []
    ones = np.ones((LK, 1), dtype=np.float32)
    for core in range(N_CORES):
        b, half = core // 2, core % 2
        q_shard = Q[b, half * LQ_SHARD : (half + 1) * LQ_SHARD, :]  # [2048, 64]
        qt = np.ascontiguousarray(q_shard.T).astype(np.float16)  # [64, 2048]
        kt = np.ascontiguousarray(K[b].T).astype(np.float16)  # [64, 4096]
        va = np.concatenate([V[b], ones], axis=1).astype(BF16NP)  # [4096, 65]
        in_maps.append({"QT": qt, "KT": kt, "VA": np.ascontiguousarray(va)})
    return in_maps


def _run(Q, K, V, trace=False, **spmd_kwargs):
    nc = _get_program()
    in_maps = _make_in_maps(Q, K, V)
    res = run_bass_kernel_spmd(
        nc, in_maps, list(range(N_CORES)), trace=trace, **spmd_kwargs
    )
    out = np.empty((B, LQ, D), dtype=np.float32)
    for core in range(N_CORES):
        b, half = core // 2, core % 2
        ot = res.results[core]["OT"]  # [64, 2048]
        out[b, half * LQ_SHARD : (half + 1) * LQ_SHARD, :] = ot.T
    return out, res


def kernel(Q, K, V):
    out, _ = _run(Q, K, V, trace=False)
    return out



# revision 2
# speedup vs baseline: 1.0012x; 1.0012x over previous
"""Cross-attention kernel for Trainium2 (8 NeuronCores, SPMD data-parallel).

Problem: O = softmax(Q @ K^T) @ V with B=4, Lq=Lk=4096, D=64, fp32 (no
1/sqrt(d) scaling).

Sharding: 8 cores = 4 batches x 2 Lq-halves. Each core handles a
[2048, 64] Q shard against the full [4096, 64] K/V of its batch.
Independent outputs -> no collectives.

Per-core algorithm (layouts chosen so nothing is transposed on-chip):
  - Host supplies QT [64, 2048] / KT [64, 4096] in fp16 (D on partitions),
    duplicated on-chip across both partition halves so two k-chunks'
    score matmuls run concurrently in the PE array via row tiling
    (contraction is only 64 rows deep).
  - ST[k, q] = matmul(lhsT=KT chunk [64,128], rhs=QT [64,512]) -> PSUM.
  - PT = exp(ST) on the scalar engine, written as bf16 (no max
    subtraction: |scores| < ~50, exp fits fp32/bf16 range; fp16 P would
    underflow).  The scalar engine at 1 elem/cycle/lane is the kernel's
    bottleneck, so exp instructions are kept at 1024 free elements.
  - OT[65, q] += matmul(lhsT=VA chunk [128, 65] bf16, rhs=PT [128, 512]):
    VA = concat([V, ones], 1); rows 0..63 accumulate unnormalized output,
    row 64 the softmax denominator. PT is consumed directly as lhsT-free
    rhs - no transpose anywhere.
  - Normalize: fast-reciprocal of row 64, gpsimd partition-broadcast,
    multiply, DMA out OT [64, 2048]; host transposes back.
"""

import sys

for _p in ("/opt/trn_rl_repo", "/opt/pypackages"):
    if _p not in sys.path:
        sys.path.insert(0, _p)

from contextlib import ExitStack

import ml_dtypes
import numpy as np

import concourse.bacc as bacc
import concourse.mybir as mybir
import concourse.tile as tile
from concourse.bass_utils import run_bass_kernel_spmd

# Problem constants (hardcoded per contract).
B, LQ, LK, D = 4, 4096, 4096, 64
N_CORES = 8
LQ_SHARD = LQ * B // N_CORES  # 2048
QB = 1024  # q-block (exp instruction free-size; 2 PSUM banks)
NQB = LQ_SHARD // QB  # 2
KC = 128  # k-chunk (contraction tile for the PV matmul)
NKC = LK // KC  # 32
SL = 512  # matmul moving-dim slice (one PSUM bank)
NSL = QB // SL  # 2

F32 = mybir.dt.float32
F16 = mybir.dt.float16
BF16 = mybir.dt.bfloat16

BF16NP = ml_dtypes.bfloat16

PACK_S = False  # row-tile two k-chunks' score matmuls concurrently
FAST_RECIP = True  # approx+NR reciprocal (~2 ULP) instead of exact (~6.5us)

KT_PIECE = 512  # kt DMA piece width (cols); 4 k-chunks per piece
VA_PIECE = 8  # va DMA piece size in k-chunks


def _build_program():
    nc = bacc.Bacc(
        "TRN2",
        target_bir_lowering=False,
        debug=False,
        num_devices=N_CORES,
    )
    qt_d = nc.declare_dram_parameter("QT", [D, LQ_SHARD], F16, isOutput=False)
    kt_d = nc.declare_dram_parameter("KT", [D, LK], F16, isOutput=False)
    va_d = nc.declare_dram_parameter("VA", [LK, D + 1], BF16, isOutput=False)
    ot_d = nc.declare_dram_parameter("OT", [D, LQ_SHARD], F32, isOutput=True)

    with tile.TileContext(nc) as tc, ExitStack() as ctx:
        singles = ctx.enter_context(tc.tile_pool(name="singles", bufs=1))
        st_pool = ctx.enter_context(tc.tile_pool(name="st", bufs=2, space="PSUM"))
        ot_pool = ctx.enter_context(tc.tile_pool(name="ot", bufs=2, space="PSUM"))
        pt_pool = ctx.enter_context(tc.tile_pool(name="pt", bufs=3))
        out_pool = ctx.enter_context(tc.tile_pool(name="out", bufs=2))
        norm_pool = ctx.enter_context(tc.tile_pool(name="norm", bufs=4))

        # Preload the exp activation table while input DMAs run.
        warm = singles.tile([1, 2], F32)
        nc.vector.memset(warm[:, :], 0.0)
        nc.scalar.activation(
            out=warm[:, :], in_=warm[:, :],
            func=mybir.ActivationFunctionType.Exp,
        )

        # QT/KT duplicated across both partition halves for PE row tiling.
        # Inputs are split into halves (separate tiles) so the first score
        # matmuls don't wait for the full 2 MB of loads; keeping the piece
        # count low preserves the Tile scheduler's pairing of the row-tiled
        # matmuls (many small tiles reorder the PE stream and let HAM
        # re-throttle the PE clock).
        va_r = va_d[:, :].rearrange("(c p) d -> p c d", p=KC)
        KH = LK // 2  # kt half width
        VH = NKC // 2  # va half size in chunks
        kt_sb = []
        qt_sb = []
        va_sb = []
        for h in range(2):
            tq = singles.tile([2 * D, QB], F16, name=f"qt{h}")
            sq = slice(h * QB, (h + 1) * QB)
            nc.sync.dma_start(out=tq[0:D, :], in_=qt_d[:, sq])
            nc.sync.dma_start(out=tq[D : 2 * D, :], in_=qt_d[:, sq])
            qt_sb.append(tq)
            t = singles.tile([2 * D, KH], F16, name=f"kt{h}")
            sl = slice(h * KH, (h + 1) * KH)
            nc.sync.dma_start(out=t[0:D, :], in_=kt_d[:, sl])
            nc.sync.dma_start(out=t[D : 2 * D, :], in_=kt_d[:, sl])
            kt_sb.append(t)
            tv = singles.tile([KC, VH, D + 1], BF16, name=f"va{h}")
            nc.sync.dma_start(
                out=tv[:, :, :], in_=va_r[:, h * VH : (h + 1) * VH, :]
            )
            va_sb.append(tv)

        def kt_ap(half, c):
            # [64, 128] fp16 weights for chunk c from partition half `half`
            t = kt_sb[c * KC // KH]
            off = (c * KC) % KH
            return t[half * D : (half + 1) * D, off : off + KC]

        def va_ap(c):
            return va_sb[c // VH][:, c % VH, :]

        for qb in range(NQB):
            ot_ps = ot_pool.tile([D + 1, QB], F32)
            for cp in range(NKC // 2):  # chunk pairs, row-tiled in the PE
                c0, c1 = 2 * cp, 2 * cp + 1
                st_a = st_pool.tile([KC, QB], F32, tag="st")
                st_b = st_pool.tile([KC, QB], F32, tag="st")
                for s in range(NSL):
                    q0 = qb * QB + s * SL
                    qt = qt_sb[qb]
                    if PACK_S:
                        nc.tensor.matmul(
                            out=st_a[:, s * SL : (s + 1) * SL],
                            lhsT=kt_ap(0, c0),
                            rhs=qt[0:D, s * SL : (s + 1) * SL],
                            start=True,
                            stop=True,
                            tile_position=(0, 0),
                        )
                        nc.tensor.matmul(
                            out=st_b[:, s * SL : (s + 1) * SL],
                            lhsT=kt_ap(1, c1),
                            rhs=qt[D : 2 * D, s * SL : (s + 1) * SL],
                            start=True,
                            stop=True,
                            tile_position=(D, 0),
                        )
                    else:
                        nc.tensor.matmul(
                            out=st_a[:, s * SL : (s + 1) * SL],
                            lhsT=kt_ap(0, c0),
                            rhs=qt[0:D, s * SL : (s + 1) * SL],
                            start=True,
                            stop=True,
                        )
                        nc.tensor.matmul(
                            out=st_b[:, s * SL : (s + 1) * SL],
                            lhsT=kt_ap(0, c1),
                            rhs=qt[0:D, s * SL : (s + 1) * SL],
                            start=True,
                            stop=True,
                        )
                for c, st_ps in ((c0, st_a), (c1, st_b)):
                    pt = pt_pool.tile([KC, QB], BF16)
                    nc.scalar.activation(
                        out=pt[:, :],
                        in_=st_ps[:, :],
                        func=mybir.ActivationFunctionType.Exp,
                    )
                    for s in range(NSL):
                        nc.tensor.matmul(
                            out=ot_ps[:, s * SL : (s + 1) * SL],
                            lhsT=va_ap(c),
                            rhs=pt[:, s * SL : (s + 1) * SL],
                            start=(c == 0),
                            stop=(c == NKC - 1),
                        )
            # Normalize: O[d, q] = OT[d, q] / OT[64, q]
            recip = norm_pool.tile([1, QB], F32)
            if FAST_RECIP:
                den = norm_pool.tile([1, QB], F32)
                nc.vector.tensor_copy(den[:, :], ot_ps[D : D + 1, :])
                scratch = norm_pool.tile([1, QB], F32)
                nc.vector.reciprocal_approx_accurate(
                    recip[:, :], den[:, :], scratch[:, :]
                )
            else:
                nc.vector.reciprocal(out=recip[:, :], in_=ot_ps[D : D + 1, :])
            bcast = norm_pool.tile([D, QB], F32)
            nc.gpsimd.partition_broadcast(bcast[:, :], recip[:, :])
            o_sb = out_pool.tile([D, QB], F32)
            nc.vector.tensor_mul(o_sb[:, :], ot_ps[0:D, :], bcast[:, :])
            nc.sync.dma_start(
                out=ot_d[:, qb * QB : (qb + 1) * QB], in_=o_sb[:, :]
            )

    nc.finalize()
    return nc


_PROGRAM_CACHE = {}


def _get_program():
    if "nc" not in _PROGRAM_CACHE:
        _PROGRAM_CACHE["nc"] = _build_program()
    return _PROGRAM_CACHE["nc"]


def _make_in_maps(Q, K, V):
    Q = np.asarray(Q, dtype=np.float32)
    K = np.asarray(K, dtype=np.float32)
    V = np.asarray(V, dtype=np.float32)
    in_maps = []
    ones = np.ones((LK, 1), dtype=np.float32)
    for core in range(N_CORES):
        b, half = core // 2, core % 2
        q_shard = Q[b, half * LQ_SHARD : (half + 1) * LQ_SHARD, :]  # [2048, 64]
        qt = np.ascontiguousarray(q_shard.T).astype(np.float16)  # [64, 2048]
        kt = np.ascontiguousarray(K[b].T).astype(np.float16)  # [64, 4096]
        va = np.concatenate([V[b], ones], axis=1).astype(BF16NP)  # [4096, 65]
        in_maps.append({"QT": qt, "KT": kt, "VA": np.ascontiguousarray(va)})
    return in_maps


def _run(Q, K, V, trace=False, **spmd_kwargs):
    nc = _get_program()
    in_maps = _make_in_maps(Q, K, V)
    res = run_bass_kernel_spmd(
        nc, in_maps, list(range(N_CORES)), trace=trace, **spmd_kwargs
    )
    out = np.empty((B, LQ, D), dtype=np.float32)
    for core in range(N_CORES):
        b, half = core // 2, core % 2
        ot = res.results[core]["OT"]  # [64, 2048]
        out[b, half * LQ_SHARD : (half + 1) * LQ_SHARD, :] = ot.T
    return out, res


def kernel(Q, K, V):
    out, _ = _run(Q, K, V, trace=False)
    return out



# revision 5
# speedup vs baseline: 1.2769x; 1.2753x over previous
"""Cross-attention kernel for Trainium2 (8 NeuronCores, SPMD data-parallel).

Problem: O = softmax(Q @ K^T) @ V with B=4, Lq=Lk=4096, D=64, fp32 (no
1/sqrt(d) scaling).

Sharding: 8 cores = 4 batches x 2 Lq-halves. Each core handles a
[2048, 64] Q shard against the full [4096, 64] K/V of its batch.
Independent outputs -> no collectives.

Per-core algorithm (HW model: the PE output bus serializes matmuls at 128
results/cycle, so row-tiled pairing gains nothing; the PE clock ramps
0.65 -> 1.2 -> 2.4 GHz only while the matmul stream is continuously busy.
The kernel is built so the PE never waits):
  - ST[k, q] = matmul(lhsT=KT chunk [64,128], rhs=QT [64,512]) -> PSUM
    [128, 1024] tiles, 3 PSUM tiles deep so scores run 3 chunks ahead.
  - exp alternates engines so neither is the bottleneck:
      even chunks: scalar ACTIVATE exact exp -> bf16
      odd  chunks: DVE Schraudolph bit-trick exp: round(x*2^7/ln2 +
      (127*128 - 7.7)) as int16, bitcast bf16 (~3% systematic rel err on
      those chunks; final output rel-L2 ~5.4e-3, tol 2e-2).
  - OT[65, q] += matmul(lhsT=VA chunk [128, 65] bf16, rhs=PT [128, 512]):
    VA = concat([V, ones], 1); rows 0..63 accumulate unnormalized output,
    row 64 the softmax denominator (exp source consistent per chunk).
  - Normalize: OT -> SBUF copy (frees PSUM fast for the next q-block),
    fast-reciprocal of row 64, gpsimd partition-broadcast, multiply, DMA
    out OT [64, 2048] f32; host transposes back.
"""

import sys

for _p in ("/opt/trn_rl_repo", "/opt/pypackages"):
    if _p not in sys.path:
        sys.path.insert(0, _p)

from contextlib import ExitStack

import ml_dtypes
import numpy as np

import concourse.bacc as bacc
import concourse.mybir as mybir
import concourse.tile as tile
from concourse.bass_utils import run_bass_kernel_spmd

# Problem constants (hardcoded per contract).
B, LQ, LK, D = 4, 4096, 4096, 64
N_CORES = 8
LQ_SHARD = LQ * B // N_CORES  # 2048
QB = 1024  # q-block (exp instruction free-size; 2 PSUM banks)
NQB = LQ_SHARD // QB  # 2
KC = 128  # k-chunk (contraction tile for the PV matmul)
NKC = LK // KC  # 32
SL = 512  # matmul moving-dim slice (one PSUM bank)
NSL = QB // SL  # 2

F32 = mybir.dt.float32
F16 = mybir.dt.float16
BF16 = mybir.dt.bfloat16
I16 = mybir.dt.int16

BF16NP = ml_dtypes.bfloat16

# Schraudolph exp -> bf16 bit pattern: round(x * 2^7/ln2 + 127*128 - C).
EXP_A = 128.0 / float(np.log(2.0))
EXP_C = 7.7
EXP_B = 127.0 * 128.0 - EXP_C

KT_PIECE = 512  # kt DMA piece width (4 k-chunks)
VA_PIECE = 8  # va DMA piece size in k-chunks
ST_BUFS = 3
PT_BUFS = 4


def _build_program():
    nc = bacc.Bacc(
        "TRN2",
        target_bir_lowering=False,
        debug=False,
        num_devices=N_CORES,
    )
    qt_d = nc.declare_dram_parameter("QT", [D, LQ_SHARD], F16, isOutput=False)
    kt_d = nc.declare_dram_parameter("KT", [D, LK], F16, isOutput=False)
    va_d = nc.declare_dram_parameter("VA", [LK, D + 1], BF16, isOutput=False)
    ot_d = nc.declare_dram_parameter("OT", [D, LQ_SHARD], F32, isOutput=True)

    with tile.TileContext(nc) as tc, ExitStack() as ctx:
        singles = ctx.enter_context(tc.tile_pool(name="singles", bufs=1))
        st_pool = ctx.enter_context(
            tc.tile_pool(name="st", bufs=ST_BUFS, space="PSUM")
        )
        ot_pool = ctx.enter_context(tc.tile_pool(name="ot", bufs=1, space="PSUM"))
        pt_pool = ctx.enter_context(tc.tile_pool(name="pt", bufs=PT_BUFS))
        osb_pool = ctx.enter_context(tc.tile_pool(name="osb", bufs=2))
        out_pool = ctx.enter_context(tc.tile_pool(name="out", bufs=4))
        norm_pool = ctx.enter_context(tc.tile_pool(name="norm", bufs=8))

        # Preload the exp activation table while input DMAs run.
        warm = singles.tile([1, 2], F32)
        nc.vector.memset(warm[:, :], 0.0)
        nc.scalar.activation(
            out=warm[:, :], in_=warm[:, :],
            func=mybir.ActivationFunctionType.Exp,
        )

        qt_sb = singles.tile([D, LQ_SHARD], F16, name="qt")
        kt_sb = singles.tile([D, LK], F16, name="kt")
        va_sb = singles.tile([KC, NKC, D + 1], BF16, name="va")
        va_r = va_d[:, :].rearrange("(c p) d -> p c d", p=KC)

        # Input DMAs, issued in consumption order (kt chunks gate the
        # score matmuls; va chunks the PV matmuls; qt half 1 only needed
        # at the second q-block).
        def dma_kt(j):
            sl = slice(j * KT_PIECE, (j + 1) * KT_PIECE)
            nc.sync.dma_start(out=kt_sb[:, sl], in_=kt_d[:, sl])

        def dma_qt(h):
            sl = slice(h * QB, (h + 1) * QB)
            nc.sync.dma_start(out=qt_sb[:, sl], in_=qt_d[:, sl])

        def dma_va(h):
            sl = slice(h * VA_PIECE, (h + 1) * VA_PIECE)
            nc.sync.dma_start(out=va_sb[:, sl, :], in_=va_r[:, sl, :])

        dma_kt(0)
        dma_qt(0)
        dma_va(0)
        dma_kt(1)
        dma_kt(2)
        dma_va(1)
        dma_kt(3)
        dma_kt(4)
        dma_qt(1)
        dma_va(2)
        dma_kt(5)
        dma_kt(6)
        dma_va(3)
        dma_kt(7)

        def emit_scores(qb, c, st_ps):
            for s in range(NSL):
                sl = slice(s * SL, (s + 1) * SL)
                qsl = slice(qb * QB + s * SL, qb * QB + (s + 1) * SL)
                nc.tensor.matmul(
                    out=st_ps[:, sl],
                    lhsT=kt_sb[:, c * KC : (c + 1) * KC],
                    rhs=qt_sb[:, qsl],
                    start=True,
                    stop=True,
                )

        def emit_exp(c, st_ps, pt):
            # pt is an int16 tile; both writers produce bf16 bit patterns.
            if c % 2 == 0:
                nc.scalar.activation(
                    out=pt[:, :].bitcast(BF16),
                    in_=st_ps[:, :],
                    func=mybir.ActivationFunctionType.Exp,
                )
            else:
                nc.vector.tensor_scalar(
                    out=pt[:, :],
                    in0=st_ps[:, :],
                    scalar1=EXP_A,
                    scalar2=EXP_B,
                    op0=mybir.AluOpType.mult,
                    op1=mybir.AluOpType.add,
                )

        def emit_pv(c, pt, ot_ps):
            for s in range(NSL):
                sl = slice(s * SL, (s + 1) * SL)
                nc.tensor.matmul(
                    out=ot_ps[:, sl],
                    lhsT=va_sb[:, c, :],
                    rhs=pt[:, sl].bitcast(BF16),
                    start=(c == 0),
                    stop=(c == NKC - 1),
                )

        for qb in range(NQB):
            ot_ps = ot_pool.tile([D + 1, QB], F32)
            st_tiles = {}
            for c in range(ST_BUFS):  # prologue: scores run ahead
                st_tiles[c] = st_pool.tile([KC, QB], F32, tag="st", name="st")
                emit_scores(qb, c, st_tiles[c])
            for c in range(NKC):
                if c + ST_BUFS < NKC:
                    st_tiles[c + ST_BUFS] = st_pool.tile(
                        [KC, QB], F32, tag="st", name="st"
                    )
                    emit_scores(qb, c + ST_BUFS, st_tiles[c + ST_BUFS])
                pt = pt_pool.tile([KC, QB], I16, tag="pt", name="pt")
                emit_exp(c, st_tiles.pop(c), pt)
                emit_pv(c, pt, ot_ps)

            # Normalize O[d, q] = OT[d, q] / OT[64, q], in halves so the
            # recip/broadcast/mul/DMA chain pipelines across engines.
            if qb < NQB - 1:
                # Free the OT PSUM banks quickly for the next q-block.
                osb = osb_pool.tile([D + 1, QB], F32)
                nc.vector.tensor_copy(osb[:, :], ot_ps[:, :])
                src = osb
            else:
                src = ot_ps
            for h in range(2):
                sl = slice(h * SL, (h + 1) * SL)
                den = norm_pool.tile([1, SL], F32)
                nc.vector.tensor_copy(den[:, :], src[D : D + 1, sl])
                recip = norm_pool.tile([1, SL], F32)
                nc.vector.reciprocal_approx_fast(recip[:, :], den[:, :])
                bcast = norm_pool.tile([D, SL], F32)
                nc.gpsimd.partition_broadcast(bcast[:, :], recip[:, :])
                o_sb = out_pool.tile([D, SL], F32)
                nc.vector.tensor_mul(o_sb[:, :], src[0:D, sl], bcast[:, :])
                osl = slice(qb * QB + h * SL, qb * QB + (h + 1) * SL)
                nc.sync.dma_start(out=ot_d[:, osl], in_=o_sb[:, :])

    nc.finalize()
    return nc


_PROGRAM_CACHE = {}


def _get_program():
    if "nc" not in _PROGRAM_CACHE:
        _PROGRAM_CACHE["nc"] = _build_program()
    return _PROGRAM_CACHE["nc"]


def _make_in_maps(Q, K, V):
    Q = np.asarray(Q, dtype=np.float32)
    K = np.asarray(K, dtype=np.float32)
    V = np.asarray(V, dtype=np.float32)
    in_maps = []
    ones = np.ones((LK, 1), dtype=np.float32)
    for core in range(N_CORES):
        b, half = core // 2, core % 2
        q_shard = Q[b, half * LQ_SHARD : (half + 1) * LQ_SHARD, :]  # [2048, 64]
        qt = np.ascontiguousarray(q_shard.T).astype(np.float16)  # [64, 2048]
        kt = np.ascontiguousarray(K[b].T).astype(np.float16)  # [64, 4096]
        va = np.concatenate([V[b], ones], axis=1).astype(BF16NP)  # [4096, 65]
        in_maps.append({"QT": qt, "KT": kt, "VA": np.ascontiguousarray(va)})
    return in_maps


def _run(Q, K, V, trace=False, **spmd_kwargs):
    nc = _get_program()
    in_maps = _make_in_maps(Q, K, V)
    res = run_bass_kernel_spmd(
        nc, in_maps, list(range(N_CORES)), trace=trace, **spmd_kwargs
    )
    out = np.empty((B, LQ, D), dtype=np.float32)
    for core in range(N_CORES):
        b, half = core // 2, core % 2
        ot = res.results[core]["OT"]  # [64, 2048]
        out[b, half * LQ_SHARD : (half + 1) * LQ_SHARD, :] = ot.T
    return out, res


def kernel(Q, K, V):
    out, _ = _run(Q, K, V, trace=False)
    return out
